# revision 47
# baseline (speedup 1.0000x reference)
"""Trainium2 Bass kernel for nn_Attention_66907000537586 (v2).

Module: x -> 1x1conv+BN (Q,K,V) -> 8-head attention with relative position
bias -> exact GELU -> 1x1conv+bias+BN.  Shapes: B=8, C=256, F=32 (n=1024
tokens), H=8, DK=32, DV=64.

Sharding: pure data-parallel over batch (one batch element per NeuronCore).

v2 schedule (vs the v1 baseline at ~185us):
  * x is cast to bf16 on host; input DMAs are split across the SP and ACT
    HWDGE rings (plus gpsimd SWDGE for the bias table) so the first
    projection starts ~10us earlier.
  * heads 0-1 Q/K are projected *directly* in 4x row-group-replicated form
    (host-replicated weight columns), so head-0 attention starts without
    waiting for the DRAM-bounce replication; heads 2-7 still use the
    compact-projection + DRAM-bounce broadcast path.
  * the per-head attention is software-pipelined with a one-head lag:
    PE stream is [dots(h) jj-pair | OU(h-1) pair | ...] so ScalarE's exp
    stream (the true bottleneck, ~64 x 1.06us) is never starved and the
    PE never idles long enough to re-throttle (HAM).
  * softmax denominators: the V^T ones-column yields S in PSUM row 64;
    1/S comes from vector.reciprocal straight out of PSUM and is
    partition-broadcast with a tiny fp32 ones-matmul on the PE --
    no DRAM round-trips in the normalize path.
  * BN offsets for V and the output conv bias/BN offset are folded into
    the matmuls as K=1 ones-row matmuls; evacuations become plain copies.
  * a couple of et2 multiplies per head run on the (otherwise idle)
    GpSimd/Pool engine to keep the DVE under the ScalarE period.
  * exact GELU is batched once at the end (single act-table switch);
    output is written bf16 and upcast on host.
"""

import numpy as np
import ml_dtypes

HEADS, DK, DV, F = 8, 32, 64, 32
C = 256
N = F * F            # 1024 tokens
B = 8
EPS = 1e-5
IDK = HEADS * DK     # 256
IDV = HEADS * DV     # 512
VTW = HEADS * (DV + 1)   # 520
SW = 2112            # per-head width of the shifted compact bias table
NJT = N // 128       # 8 j-tiles
NIT = N // 512       # 2 i-tiles
WSEG_D4 = 256        # wA/wB layout offsets (qk-m1 at 0, then d4 h0-3, v, wo)
WSEG_V = 1280
WSEG_WO = 1800
WAF = 2312           # total combined-weight free width

# which j's et2 multiply runs on the Pool (gpsimd) engine.  The steady-state
# DVE budget (6 muls + normalize phases) just fits the 8.6us ScalarE period;
# two multiplies per head go to the otherwise-idle Pool engine.  The Pool
# queue must stay free of mid-stream DMA triggers (strictly in-order!) --
# all its SWDGE input DMAs are issued up front.
POOL_JS = (2, 5)

_PROGRAM_CACHE = {}


def _split_excess_waits(nc, mybir, limit=1):
    """Two post-passes over the scheduled BIR:

    1. Drop PE->PE self-semaphore waits from PE instructions (they defeat
       tile_position row-group concurrency; every PSUM-slot reuse is
       already guarded by the consumer engine's wait).
    2. Move excess semaphore sync-waits (>limit) onto carrier NoOps."""
    k = 0
    for fn in nc.m.functions:
        for bb in fn.blocks:
            out = []
            for inst in bb.instructions:
                si = inst.sync_info
                if (si is not None and si.on_wait
                        and str(inst.engine) == "EngineType.PE"
                        and type(inst).__name__ in ("InstMatmult", "InstLdweights")):
                    kept = [w for w in si.on_wait
                            if not str(w.ant_name).startswith("PE_")]
                    if len(kept) != len(si.on_wait):
                        si.on_wait = kept
                waits = list(si.on_wait) if si is not None else []
                if len(waits) > limit:
                    extra, keep = waits[:-limit], waits[-limit:]
                    for i in range(0, len(extra), limit):
                        nop = mybir.InstNoOp(name=f"waitsplit_{k}")
                        k += 1
                        nop.engine = inst.engine
                        nop.sync_info = mybir.SyncInfo(
                            on_wait=extra[i:i + limit], on_update=[])
                        out.append(nop)
                    si.on_wait = keep
                out.append(inst)
            bb.instructions = out


def build_program(structured=True):
    """Build the single-core Bass program (run SPMD on 8 cores)."""
    import concourse.bass as bass
    import concourse.mybir as mybir
    import concourse.tile as tile

    dt = mybir.dt
    nc = bass.Bass("TRN2", target_bir_lowering=False, debug=False, num_devices=B)

    f32, bf16 = dt.float32, dt.bfloat16
    Ident = mybir.ActivationFunctionType.Identity
    Exp = mybir.ActivationFunctionType.Exp
    Gelu = mybir.ActivationFunctionType.Gelu

    x = nc.dram_tensor("x", [C, N], bf16, kind="ExternalInput")
    wA = nc.dram_tensor("wA", [128, WAF], bf16, kind="ExternalInput")
    wB = nc.dram_tensor("wB", [128, WAF], bf16, kind="ExternalInput")
    offs = nc.dram_tensor("offs", [128, 12], f32, kind="ExternalInput")
    rowc = nc.dram_tensor("rowc", [1, VTW + C], bf16, kind="ExternalInput")
    if structured:
        sst = nc.dram_tensor("sst", [128, HEADS * SW], bf16, kind="ExternalInput")
    else:
        sst = nc.dram_tensor("sst", [HEADS * NJT * 128, N], bf16, kind="ExternalInput")
    out = nc.dram_tensor("out", [C, N], bf16, kind="ExternalOutput")

    with tile.TileContext(nc) as tc:
        with (
            tc.tile_pool(name="persist", bufs=1) as pp,
            tc.tile_pool(name="exps", bufs=8) as ep,
            tc.tile_pool(name="exps2", bufs=12) as e2p,
            tc.tile_pool(name="norm", bufs=2) as np_pool,
            tc.tile_pool(name="bias_stream", bufs=4) as bp,
            tc.tile_pool(name="dramscratch", bufs=2, space="DRAM") as dp,
            tc.tile_pool(name="ps8", bufs=2, space="PSUM") as ps8,
        ):
            # ---- input DMAs.  SP ring: x then bounce/normalize traffic.
            # ACT ring: offsets + weights, direct-proj (d4) segment first.
            # gpsimd SWDGE: bias table + h4-7 replication (off both rings).
            xbf = [pp.tile([128, N], bf16, tag=f"xbf{k}", name=f"xbf{k}")
                   for k in range(2)]
            nc.sync.dma_start(out=xbf[0], in_=x.ap()[0:128, :])
            nc.sync.dma_start(out=xbf[1][:, 0:512], in_=x.ap()[128:256, 0:512])
            rowcsb = pp.tile([1, VTW + C], bf16, tag="rowc")
            nc.sync.dma_start(out=rowcsb, in_=rowc.ap())

            # Weights live in per-segment tiles: tile-granular DMA
            # dependencies mean a consumer must not wait for unrelated
            # segments still in flight.
            wqk = [pp.tile([128, 256], bf16, tag=f"wqk{k}", name=f"wqk{k}")
                   for k in range(2)]
            wd4a = [pp.tile([128, 512], bf16, tag=f"wd4a{k}", name=f"wd4a{k}")
                    for k in range(2)]
            wd4b = [pp.tile([128, 512], bf16, tag=f"wd4b{k}", name=f"wd4b{k}")
                    for k in range(2)]
            wv = [pp.tile([128, VTW], bf16, tag=f"wv{k}", name=f"wv{k}")
                  for k in range(2)]
            wwo = [pp.tile([128, 512], bf16, tag=f"wwo{k}", name=f"wwo{k}")
                   for k in range(2)]
            offssb = pp.tile([128, 12], f32, tag="offs")
            nc.scalar.dma_start(out=offssb, in_=offs.ap())
            for k in range(2):  # direct (d4) weights heads 0-1 first
                nc.scalar.dma_start(
                    out=wd4a[k],
                    in_=(wA if k == 0 else wB).ap()[:, WSEG_D4:WSEG_D4 + 512])
            # second half of x chunk 1 rides the ACT ring
            nc.scalar.dma_start(out=xbf[1][:, 512:N],
                                in_=x.ap()[128:256, 512:N])

            ones_bf = pp.tile([1, 512], bf16, tag="ones_bf")
            nc.vector.memset(ones_bf, 1.0)
            # tiny dummy exp: hoists the exp act-table load to kernel start
            tbl = np_pool.tile([1, 8], f32, tag="tbl", name="tbl")
            nc.vector.memset(tbl, 0.0)
            nc.scalar.activation(tbl, tbl, Exp)

            for k in range(2):  # direct (d4) weights heads 2-3
                nc.scalar.dma_start(
                    out=wd4b[k],
                    in_=(wA if k == 0 else wB).ap()[:, WSEG_D4 + 512:WSEG_V])
            for k in range(2):  # compact q/k (m=1) weight segments
                nc.scalar.dma_start(
                    out=wqk[k], in_=(wA if k == 0 else wB).ap()[:, 0:WSEG_D4])

            sstsb = None
            if structured:  # four head-pair tiles on the gpsimd SWDGE ring
                sstsb = [pp.tile([128, 2 * SW], bf16, tag=f"sst{g}",
                                 name=f"sst{g}") for g in range(4)]
                nc.gpsimd.dma_start(out=sstsb[0],
                                    in_=sst.ap()[:, 0:2 * SW])
            for k in range(2):  # V weight segment (needed from ~t+20us)
                nc.gpsimd.dma_start(
                    out=wv[k], in_=(wA if k == 0 else wB).ap()[:, WSEG_V:WSEG_WO])
            if structured:
                for g in range(1, 4):
                    nc.gpsimd.dma_start(
                        out=sstsb[g],
                        in_=sst.ap()[:, 2 * g * SW:2 * (g + 1) * SW])
            for k in range(2):  # wo weight segment (needed only for the tail)
                nc.gpsimd.dma_start(
                    out=wwo[k], in_=(wA if k == 0 else wB).ap()[:, WSEG_WO:])

            # PE warmup: dummy matmuls ramp the HAM clock gate to full speed
            # and keep it warm (no >5us idle) until the input DMAs land.
            # The tile shares the "ou" tag/banks (disjoint lifetime).
            warm = ps8.tile([128, 512], f32, tag="ou", bufs=4, name="warm")
            for _ in range(10):
                nc.tensor.matmul(warm, lhsT=ones_bf[:, 0:128],
                                 rhs=ones_bf[:, 0:512], start=True, stop=True)

            # ---- compact Q/K projections + DRAM-bounce replication (h2-7),
            # direct replicated projection for heads 0-1 ----
            qsb = [pp.tile([128, N], bf16, tag=f"qsb{h}", name=f"qsb{h}")
                   for h in range(HEADS)]
            ksb = [pp.tile([128, N], bf16, tag=f"ksb{h}", name=f"ksb{h}")
                   for h in range(HEADS)]
            qcomp = pp.tile([128, N], bf16, tag="qc", name="qc")
            kcomp = pp.tile([128, N], bf16, tag="kc", name="kc")
            qdram = dp.tile([128, N], bf16, tag="qd", bufs=1, name="qd")
            kdram = dp.tile([128, N], bf16, tag="kd", bufs=1, name="kd")

            def emit_compact_proj():
                # m=1 chunk only (heads 4-7); heads 0-3 project directly
                for ten in range(2):  # 0 = Q, 1 = K
                    ps = ps8.tile([128, N], f32, tag="ps", name="ps")
                    for nt in range(NIT):
                        for k in range(2):
                            nc.tensor.matmul(
                                ps[:, nt * 512:(nt + 1) * 512],
                                lhsT=wqk[k][:, ten * 128:(ten + 1) * 128],
                                rhs=xbf[k][:, nt * 512:(nt + 1) * 512],
                                start=(k == 0), stop=(k == 1),
                            )
                    if ten == 0:  # Q on ScalarE, K on VectorE
                        nc.scalar.activation(qcomp, ps, Ident,
                                             bias=offssb[:, 0:1])
                        nc.sync.dma_start(out=qdram, in_=qcomp)
                    else:
                        nc.vector.tensor_scalar_add(kcomp, ps,
                                                    offssb[:, 1:2])
                        nc.sync.dma_start(out=kdram, in_=kcomp)

            def emit_direct_proj(h):
                """4x row-group-replicated Q/K projection for head h."""
                wd4 = wd4a if h < 2 else wd4b
                for ten in range(2):  # 0 = Q, 1 = K
                    ps = ps8.tile([128, N], f32, tag="ps", name="ps")
                    base = (2 * (h % 2) + ten) * 128
                    for nt in range(NIT):
                        for k in range(2):
                            nc.tensor.matmul(
                                ps[:, nt * 512:(nt + 1) * 512],
                                lhsT=wd4[k][:, base:base + 128],
                                rhs=xbf[k][:, nt * 512:(nt + 1) * 512],
                                start=(k == 0), stop=(k == 1),
                            )
                    col = 2 + 2 * h + ten
                    if ten == 0:
                        nc.scalar.activation(qsb[h], ps, Ident,
                                             bias=offssb[:, col:col + 1])
                    else:
                        nc.vector.tensor_scalar_add(ksb[h], ps,
                                                    offssb[:, col:col + 1])

            def emit_replication(h, engine):
                for (dram, dst4) in ((qdram, qsb), (kdram, ksb)):
                    engine.dma_start(
                        out=dst4[h],
                        in_=bass.AP(tensor=dram.tensor,
                                    offset=dram.offset + (h - 4) * 32 * N,
                                    ap=[[0, 4], [N, 32], [1, N]]),
                    )

            # ---- attention: one-head-lag software pipeline ----
            vtsb = [pp.tile([128, VTW], bf16, tag=f"vtsb{j}", name=f"vtsb{j}")
                    for j in range(NJT)]
            gsb = [pp.tile([128, N], bf16, tag=f"gsb{t}", name=f"gsb{t}")
                   for t in range(4)]

            ets = {}          # (h, j) -> et2 tile
            ous = {}          # h -> {it: psum tile}
            state = {"last_exp": None}

            def emit_dots_pair(h, jj):
                """4 row-group-packed dots matmuls for j-tiles jj, jj+1."""
                dts = {}
                bts = {}
                for sj in range(2):
                    j = jj + sj
                    dts[j] = ps8.tile([128, N], f32, tag="ps",
                                      name=f"dots{h}_{j}")
                    if not structured:
                        bts[j] = bp.tile([128, N], bf16, tag="bt", name="bt")
                        base = (h * NJT + j) * 128
                        nc.sync.dma_start(out=bts[j],
                                          in_=sst.ap()[base:base + 128, :])
                for sj in range(2):
                    j = jj + sj
                    for it in range(NIT):
                        sl = slice(it * 512, (it + 1) * 512)
                        rg = 32 * ((2 * j + it) % 4)
                        nc.tensor.matmul(
                            dts[j][:, sl],
                            lhsT=ksb[h][rg:rg + 32, j * 128:(j + 1) * 128],
                            rhs=qsb[h][rg:rg + 32, sl],
                            start=True, stop=True,
                            tile_position=(rg, 0),
                        )
                for sj in range(2):
                    j = jj + sj
                    et = ep.tile([128, N], bf16, tag="et", name="et")
                    state["last_exp"] = nc.scalar.activation(et, dts[j], Exp)
                    et2 = e2p.tile([128, N], bf16, tag="et2", name="et2")
                    if structured:
                        off = (h % 2) * SW + (31 - 4 * j) * 32
                        bias_ap = sstsb[h // 2][:, off:off + N]
                    else:
                        bias_ap = bts[j]
                    eng = nc.gpsimd if j in POOL_JS else nc.vector
                    eng.tensor_mul(et2, et, bias_ap)
                    ets[(h, j)] = et2

            def emit_ou_pair(h, jj):
                """OU accumulation matmuls for j-tiles jj, jj+1 of head h."""
                if jj == 0:
                    ous[h] = {it: ps8.tile([65, 512], f32, tag="ou", bufs=4,
                                           name=f"ou{h}_{it}")
                              for it in range(NIT)}
                for sj in range(2):
                    j = jj + sj
                    for it in range(NIT):
                        sl = slice(it * 512, (it + 1) * 512)
                        nc.tensor.matmul(
                            ous[h][it],
                            lhsT=vtsb[j][:, h * 65:h * 65 + 65],
                            rhs=ets[(h, j)][:, sl],
                            start=(j == 0), stop=(j == NJT - 1),
                        )

            rbs = {}

            def emit_normalize_a(h):
                """Launch the 1/S chain for head h: S -> SBUF (DVE copies),
                DRAM bounce into a [128, 8] gather so the DVE reciprocal is
                partition-wide (free-size 8, ~70ns -- NOT free-size 512),
                scatter back and broadcast-load.  The ~9us DMA latency is
                hidden: the G-multiply runs one block later (phase b)."""
                sx = np_pool.tile([1, N], f32, tag="sx", name="sx")
                for it in range(NIT):
                    nc.vector.tensor_copy(sx[:, it * 512:(it + 1) * 512],
                                          ous[h][it][64:65, :])
                rd = dp.tile([1, N], f32, tag="rd", name="rd")
                nc.sync.dma_start(out=rd, in_=sx)
                s8 = np_pool.tile([128, 8], f32, tag="s8", name="s8")
                nc.sync.dma_start(
                    out=s8,
                    in_=bass.AP(tensor=rd.tensor, offset=rd.offset,
                                ap=[[8, 128], [1, 8]]),
                )
                r8 = np_pool.tile([128, 8], bf16, tag="r8", name="r8")
                with nc.allow_low_precision(
                        reason="1/S in bf16: 0.4% on a 2e-2 gate"):
                    nc.vector.reciprocal(r8, s8)
                rd2 = dp.tile([1, N], bf16, tag="rd2", name="rd2")
                nc.sync.dma_start(
                    out=bass.AP(tensor=rd2.tensor, offset=rd2.offset,
                                ap=[[8, 128], [1, 8]]),
                    in_=r8,
                )
                rb = np_pool.tile([64, N], bf16, tag="rb", name="rb")
                nc.sync.dma_start(
                    out=rb,
                    in_=bass.AP(tensor=rd2.tensor, offset=rd2.offset,
                                ap=[[0, 64], [1, N]]),
                )
                rbs[h] = rb

            def emit_normalize_b(h):
                """G = OU * (1/S) into gsb; rb arrived a block ago."""
                rb = rbs.pop(h)
                for it in range(NIT):
                    sl = slice(it * 512, (it + 1) * 512)
                    nc.vector.tensor_mul(
                        gsb[h // 2][64 * (h % 2):64 * (h % 2) + 64, sl],
                        ous[h][it][0:64, :], rb[:, sl])
                del ous[h]

            def emit_v_pair(jj):
                """V^T projection j-tiles jj, jj+1; BN offsets and the
                softmax-denominator ones column folded in as a K=1
                ones-row matmul."""
                for j in (jj, jj + 1):
                    ps = ps8.tile([128, N], f32, tag="ps", name="vps")
                    for (lo, hi) in ((0, 512), (512, VTW)):
                        for k in range(2):
                            nc.tensor.matmul(
                                ps[:, lo:hi],
                                lhsT=xbf[k][:, j * 128:(j + 1) * 128],
                                rhs=wv[k][:, lo:hi],
                                start=(k == 0), stop=False,
                            )
                        nc.tensor.matmul(
                            ps[:, lo:hi],
                            lhsT=ones_bf[:, 0:128],
                            rhs=rowcsb[:, lo:hi],
                            start=False, stop=True,
                        )
                    nc.vector.tensor_copy(vtsb[j], ps[:, 0:VTW])

            # head 0 + startup: direct-proj head 0, first dots pair ASAP,
            # then slot the remaining projections (direct h1, compact m0/m1
            # for the h2-7 bounce, V pairs) behind dots pairs on the PE
            emit_direct_proj(0)
            emit_dots_pair(0, 0)
            emit_direct_proj(1)
            emit_dots_pair(0, 2)
            emit_direct_proj(2)
            emit_dots_pair(0, 4)
            emit_direct_proj(3)
            emit_compact_proj()
            for h in range(4, HEADS):
                emit_replication(h, nc.sync)
            emit_dots_pair(0, 6)
            emit_v_pair(0)
            emit_v_pair(2)
            emit_v_pair(4)
            emit_v_pair(6)

            # heads 1..7: dots(h) interleaved with OU(h-1); OU(7) trails.
            # normalize phases at block starts: b(h-3) multiplies with the
            # rb that arrived last block; a(h-2) launches the next chain.
            for h in range(1, HEADS):
                if h >= 3:
                    emit_normalize_b(h - 3)
                if h >= 2:
                    emit_normalize_a(h - 2)
                for jj in range(0, NJT, 2):
                    emit_dots_pair(h, jj)
                    emit_ou_pair(h - 1, jj)
            emit_normalize_a(HEADS - 2)
            emit_normalize_b(HEADS - 3)
            for jj in range(0, NJT, 2):
                emit_ou_pair(HEADS - 1, jj)
            emit_normalize_a(HEADS - 1)
            emit_normalize_b(HEADS - 2)
            emit_normalize_b(HEADS - 1)

            # ---- batched exact GELU (single act-table switch).  Gate the
            # first gelu on the last exp so the scheduler can't hoist a gelu
            # into a mid-stream ACT idle slot (act-table thrash). ----
            from concourse.tile import add_dep_helper
            for t in range(4):
                gi = nc.scalar.activation(gsb[t], gsb[t], Gelu)
                if state["last_exp"] is not None:
                    add_dep_helper(gi.ins, state["last_exp"].ins, sync=True,
                                   reason="single gelu table switch")

            # ---- output conv; conv bias+BN offset folded as ones-matmul.
            # k=0..2 partial sums for the first two groups run while
            # normalize(7)'s broadcast is still in flight; the k=3 matmuls
            # (gated on the last gelu) come last per group. ----
            groups = [(m, nt) for m in range(2) for nt in range(NIT)]
            cpss = {}
            for (m, nt) in groups[:2]:
                sl = slice(nt * 512, (nt + 1) * 512)
                cps = cpss[(m, nt)] = ps8.tile([128, N], f32, tag="ps",
                                               name="cps")
                for k in range(3):
                    nc.tensor.matmul(
                        cps[:, 0:512],
                        lhsT=wwo[k // 2][:, (k % 2) * 256 + m * 128:
                                         (k % 2) * 256 + (m + 1) * 128],
                        rhs=gsb[k][:, sl],
                        start=(k == 0), stop=False,
                    )
            for gi_, (m, nt) in enumerate(groups):
                sl = slice(nt * 512, (nt + 1) * 512)
                if (m, nt) in cpss:
                    cps = cpss[(m, nt)]
                else:
                    cps = ps8.tile([128, N], f32, tag="ps", name="cps")
                    for k in range(3):
                        nc.tensor.matmul(
                            cps[:, 0:512],
                            lhsT=wwo[k // 2][:, (k % 2) * 256 + m * 128:
                                             (k % 2) * 256 + (m + 1) * 128],
                            rhs=gsb[k][:, sl],
                            start=(k == 0), stop=False,
                        )
                nc.tensor.matmul(
                    cps[:, 0:512],
                    lhsT=wwo[1][:, 256 + m * 128:256 + (m + 1) * 128],
                    rhs=gsb[3][:, sl],
                    start=False, stop=False,
                )
                nc.tensor.matmul(
                    cps[:, 0:512],
                    lhsT=rowcsb[:, VTW + m * 128:VTW + (m + 1) * 128],
                    rhs=ones_bf[:, 0:512],
                    start=False, stop=True,
                )
                ysb = np_pool.tile([128, 512], bf16, tag="ysb", name="ysb")
                nc.vector.tensor_copy(ysb, cps[:, 0:512])
                nc.sync.dma_start(
                    out=out.ap()[m * 128:(m + 1) * 128, sl], in_=ysb)

    _split_excess_waits(nc, mybir)
    return nc


def _fold_inputs(inp):
    """Host-side BN/scale folding + compact bias table construction."""
    f32 = np.float32
    bfc = ml_dtypes.bfloat16
    scale = DK ** -0.5
    x = np.asarray(inp["x"], f32)

    def bn_fold(w, gam, bet, mu, var, s=1.0):
        inv = np.asarray(gam, f32) / np.sqrt(np.asarray(var, f32) + EPS)
        wf = (np.asarray(w, f32) * inv[:, None] * s).T.copy()        # (C, O)
        off = (np.asarray(bet, f32) - np.asarray(mu, f32) * inv) * s  # (O,)
        return wf, off

    wqt, qoff = bn_fold(inp["wq"], inp["qgam"], inp["qbet"], inp["qmu"],
                        inp["qvar"], scale)
    wkt, koff = bn_fold(inp["wk"], inp["kgam"], inp["kbet"], inp["kmu"],
                        inp["kvar"])
    wvt0, voff = bn_fold(inp["wv"], inp["vgam"], inp["vbet"], inp["vmu"],
                         inp["vvar"])

    wvt = np.zeros((C, VTW), f32)
    voffi = np.zeros(VTW, f32)
    for h in range(HEADS):
        wvt[:, 65 * h:65 * h + 64] = wvt0[:, 64 * h:64 * h + 64]
        voffi[65 * h:65 * h + 64] = voff[64 * h:64 * h + 64]
        voffi[65 * h + 64] = 1.0   # ones column -> softmax denominator

    oinv = np.asarray(inp["ogam"], f32) / np.sqrt(np.asarray(inp["ovar"], f32) + EPS)
    wot = (np.asarray(inp["wo"], f32) * oinv[:, None]).T.copy()       # (512, 256)
    ooff_v = (np.asarray(inp["bo"], f32) * oinv
              + np.asarray(inp["obet"], f32) - np.asarray(inp["omu"], f32) * oinv)

    # combined weight tensors wA (k-chunk 0) / wB (k-chunk 1):
    # [ q_m1 | k_m1 | d4 heads 0-3 (q4_h | k4_h) | wvt | wot_a | wot_b ]
    wAB = np.zeros((2, 128, WAF), f32)
    for k in range(2):
        ck = slice(k * 128, (k + 1) * 128)
        wAB[k, :, 0:128] = wqt[ck, 128:256]
        wAB[k, :, 128:256] = wkt[ck, 128:256]
        wAB[k, :, WSEG_V:WSEG_V + VTW] = wvt[ck]
        wAB[k, :, WSEG_WO:WSEG_WO + 256] = wot[(2 * k) * 128:(2 * k) * 128 + 128]
        wAB[k, :, WSEG_WO + 256:WSEG_WO + 512] = wot[(2 * k + 1) * 128:
                                                     (2 * k + 1) * 128 + 128]
        for h in range(4):
            wAB[k, :, WSEG_D4 + (2 * h) * 128:WSEG_D4 + (2 * h + 1) * 128] = \
                np.tile(wqt[ck, 32 * h:32 * h + 32], (1, 4))
            wAB[k, :, WSEG_D4 + (2 * h + 1) * 128:WSEG_D4 + (2 * h + 2) * 128] = \
                np.tile(wkt[ck, 32 * h:32 * h + 32], (1, 4))

    offs = np.zeros((128, 12), f32)
    offs[:, 0] = qoff[128:]
    offs[:, 1] = koff[128:]
    for h in range(4):
        offs[:, 2 + 2 * h] = np.tile(qoff[32 * h:32 * h + 32], 4)
        offs[:, 3 + 2 * h] = np.tile(koff[32 * h:32 * h + 32], 4)

    rowc = np.zeros((1, VTW + C), f32)
    rowc[0, :VTW] = voffi
    rowc[0, VTW:] = ooff_v

    pe = np.asarray(inp["pos_emb"], f32)             # (1024, 8)
    pidx = np.asarray(inp["pos_indices"])            # (1024, 1024) int32

    r = np.arange(F)
    pos = np.stack(np.meshgrid(r, r, indexing="ij"), axis=-1).reshape(-1, 2)
    rel = np.abs(pos[:, None, :] - pos[None, :, :])
    expected = (rel[..., 0] * F + rel[..., 1]).astype(pidx.dtype)
    structured = bool(np.array_equal(pidx, expected))

    if structured:
        dd = np.arange(63)
        xi_ = np.maximum(dd - 31, 0)
        xj_ = np.maximum(31 - dd, 0)
        yy = np.arange(F)
        I = xi_[:, None] * F + yy[None, :]           # (63, yi)
        J = xj_[:, None] * F + yy[None, :]           # (63, yj)
        idx = pidx[I[:, None, :], J[:, :, None]]     # (dd, yj, yi)
        sb = pe[idx] / scale                         # (dd, yj, yi, H)
        flat = np.ascontiguousarray(sb.transpose(3, 1, 0, 2)).reshape(HEADS, 32, 2016)
        eflat = np.exp(flat)   # exp(dots+bias) = exp(dots) * exp(bias)
        sstv = np.zeros((128, HEADS * SW), f32)
        for g in range(4):
            for h in range(HEADS):
                sstv[32 * g:32 * g + 32,
                     h * SW + 32 * g:h * SW + 32 * g + 2016] = eflat[h]
    else:
        biasT = np.exp(pe[pidx] / scale).transpose(2, 1, 0)  # (H, j, i)
        sstv = biasT.reshape(HEADS * NJT * 128, N)
    sstv = sstv.astype(bfc)

    common = dict(wA=wAB[0].astype(bfc), wB=wAB[1].astype(bfc),
                  offs=offs, rowc=rowc.astype(bfc), sst=sstv)
    in_maps = []
    for b in range(B):
        m = dict(common)
        m["x"] = np.ascontiguousarray(x[b].reshape(C, N)).astype(bfc)
        in_maps.append(m)
    return in_maps, structured


def run(inputs, trace=False, trace_cores=None):
    in_maps, structured = _fold_inputs(inputs)
    key = ("nc", structured)
    if key not in _PROGRAM_CACHE:
        _PROGRAM_CACHE[key] = build_program(structured)
    nc = _PROGRAM_CACHE[key]
    from concourse.bass_utils import run_bass_kernel_spmd
    res = run_bass_kernel_spmd(
        nc, in_maps, core_ids=list(range(B)), trace=trace, trace_cores=trace_cores
    )
    out = np.stack([np.asarray(res.results[b]["out"], dtype=np.float32)
                    for b in range(B)], axis=0)
    return out.reshape(B, C, F, F), res


def kernel(**inputs):
    out, _ = run(inputs, trace=False)
    return out


# revision 48
# speedup vs baseline: 1.0256x; 1.0256x over previous
"""Trainium2 Bass kernel for nn_Attention_66907000537586 (v2).

Module: x -> 1x1conv+BN (Q,K,V) -> 8-head attention with relative position
bias -> exact GELU -> 1x1conv+bias+BN.  Shapes: B=8, C=256, F=32 (n=1024
tokens), H=8, DK=32, DV=64.

Sharding: pure data-parallel over batch (one batch element per NeuronCore).

v2 schedule (vs the v1 baseline at ~185us):
  * x is cast to bf16 on host; input DMAs are split across the SP and ACT
    HWDGE rings (plus gpsimd SWDGE for the bias table) so the first
    projection starts ~10us earlier.
  * heads 0-1 Q/K are projected *directly* in 4x row-group-replicated form
    (host-replicated weight columns), so head-0 attention starts without
    waiting for the DRAM-bounce replication; heads 2-7 still use the
    compact-projection + DRAM-bounce broadcast path.
  * the per-head attention is software-pipelined with a one-head lag:
    PE stream is [dots(h) jj-pair | OU(h-1) pair | ...] so ScalarE's exp
    stream (the true bottleneck, ~64 x 1.06us) is never starved and the
    PE never idles long enough to re-throttle (HAM).
  * softmax denominators: the V^T ones-column yields S in PSUM row 64;
    1/S comes from vector.reciprocal straight out of PSUM and is
    partition-broadcast with a tiny fp32 ones-matmul on the PE --
    no DRAM round-trips in the normalize path.
  * BN offsets for V and the output conv bias/BN offset are folded into
    the matmuls as K=1 ones-row matmuls; evacuations become plain copies.
  * a couple of et2 multiplies per head run on the (otherwise idle)
    GpSimd/Pool engine to keep the DVE under the ScalarE period.
  * exact GELU is batched once at the end (single act-table switch);
    output is written bf16 and upcast on host.
"""

import numpy as np
import ml_dtypes

HEADS, DK, DV, F = 8, 32, 64, 32
C = 256
N = F * F            # 1024 tokens
B = 8
EPS = 1e-5
IDK = HEADS * DK     # 256
IDV = HEADS * DV     # 512
VTW = HEADS * (DV + 1)   # 520
SW = 2112            # per-head width of the shifted compact bias table
NJT = N // 128       # 8 j-tiles
NIT = N // 512       # 2 i-tiles
WSEG_D4 = 256        # wA/wB layout offsets (qk-m1 at 0, then d4 h0-3, v, wo)
WSEG_V = 1280
WSEG_WO = 1800
WAF = 2312           # total combined-weight free width

# NOTE: the Pool/GpSimd engine shares its SBUF ports with the DVE, so
# offloading elementwise work there degrades DVE throughput -- the bias is
# instead added in PSUM by identity-matmuls on the PE (which has slack) and
# exp produces et2 = exp(dots + bias) in a single ScalarE op, leaving the
# DVE with only the normalize phases and evacuations.

_PROGRAM_CACHE = {}


def _split_excess_waits(nc, mybir, limit=1):
    """Two post-passes over the scheduled BIR:

    1. Drop PE->PE self-semaphore waits from PE instructions (they defeat
       tile_position row-group concurrency; every PSUM-slot reuse is
       already guarded by the consumer engine's wait).
    2. Move excess semaphore sync-waits (>limit) onto carrier NoOps."""
    k = 0
    for fn in nc.m.functions:
        for bb in fn.blocks:
            out = []
            for inst in bb.instructions:
                si = inst.sync_info
                if (si is not None and si.on_wait
                        and str(inst.engine) == "EngineType.PE"
                        and type(inst).__name__ in ("InstMatmult", "InstLdweights")):
                    kept = [w for w in si.on_wait
                            if not str(w.ant_name).startswith("PE_")]
                    if len(kept) != len(si.on_wait):
                        si.on_wait = kept
                waits = list(si.on_wait) if si is not None else []
                if len(waits) > limit:
                    extra, keep = waits[:-limit], waits[-limit:]
                    for i in range(0, len(extra), limit):
                        nop = mybir.InstNoOp(name=f"waitsplit_{k}")
                        k += 1
                        nop.engine = inst.engine
                        nop.sync_info = mybir.SyncInfo(
                            on_wait=extra[i:i + limit], on_update=[])
                        out.append(nop)
                    si.on_wait = keep
                out.append(inst)
            bb.instructions = out


def build_program(structured=True):
    """Build the single-core Bass program (run SPMD on 8 cores)."""
    import concourse.bass as bass
    import concourse.mybir as mybir
    import concourse.tile as tile

    dt = mybir.dt
    nc = bass.Bass("TRN2", target_bir_lowering=False, debug=False, num_devices=B)

    f32, bf16 = dt.float32, dt.bfloat16
    Ident = mybir.ActivationFunctionType.Identity
    Exp = mybir.ActivationFunctionType.Exp
    Gelu = mybir.ActivationFunctionType.Gelu

    x = nc.dram_tensor("x", [C, N], bf16, kind="ExternalInput")
    wA = nc.dram_tensor("wA", [128, WAF], bf16, kind="ExternalInput")
    wB = nc.dram_tensor("wB", [128, WAF], bf16, kind="ExternalInput")
    offs = nc.dram_tensor("offs", [128, 12], f32, kind="ExternalInput")
    rowc = nc.dram_tensor("rowc", [1, VTW + C], bf16, kind="ExternalInput")
    if structured:
        sst = nc.dram_tensor("sst", [128, HEADS * SW], bf16, kind="ExternalInput")
    else:
        sst = nc.dram_tensor("sst", [HEADS * NJT * 128, N], bf16, kind="ExternalInput")
    out = nc.dram_tensor("out", [C, N], bf16, kind="ExternalOutput")
    ident_dram = nc.inline_tensor(np.eye(128, dtype=ml_dtypes.bfloat16),
                                  name="ident128")

    with tile.TileContext(nc) as tc:
        with (
            tc.tile_pool(name="persist", bufs=1) as pp,
            tc.tile_pool(name="exps2", bufs=12) as e2p,
            tc.tile_pool(name="norm", bufs=2) as np_pool,
            tc.tile_pool(name="bias_stream", bufs=4) as bp,
            tc.tile_pool(name="dramscratch", bufs=2, space="DRAM") as dp,
            tc.tile_pool(name="ps8", bufs=2, space="PSUM") as ps8,
        ):
            # ---- input DMAs.  SP ring: x then bounce/normalize traffic.
            # ACT ring: offsets + weights, direct-proj (d4) segment first.
            # gpsimd SWDGE: bias table + h4-7 replication (off both rings).
            xbf = [pp.tile([128, N], bf16, tag=f"xbf{k}", name=f"xbf{k}")
                   for k in range(2)]
            nc.sync.dma_start(out=xbf[0], in_=x.ap()[0:128, :])
            nc.sync.dma_start(out=xbf[1][:, 0:512], in_=x.ap()[128:256, 0:512])
            rowcsb = pp.tile([1, VTW + C], bf16, tag="rowc")
            nc.sync.dma_start(out=rowcsb, in_=rowc.ap())

            # Weights live in per-segment tiles: tile-granular DMA
            # dependencies mean a consumer must not wait for unrelated
            # segments still in flight.
            wqk = [pp.tile([128, 256], bf16, tag=f"wqk{k}", name=f"wqk{k}")
                   for k in range(2)]
            wd4a = [pp.tile([128, 512], bf16, tag=f"wd4a{k}", name=f"wd4a{k}")
                    for k in range(2)]
            wd4b = [pp.tile([128, 512], bf16, tag=f"wd4b{k}", name=f"wd4b{k}")
                    for k in range(2)]
            wv = [pp.tile([128, VTW], bf16, tag=f"wv{k}", name=f"wv{k}")
                  for k in range(2)]
            wwo = [pp.tile([128, 512], bf16, tag=f"wwo{k}", name=f"wwo{k}")
                   for k in range(2)]
            offssb = pp.tile([128, 12], f32, tag="offs")
            nc.scalar.dma_start(out=offssb, in_=offs.ap())
            for k in range(2):  # direct (d4) weights heads 0-1 first
                nc.scalar.dma_start(
                    out=wd4a[k],
                    in_=(wA if k == 0 else wB).ap()[:, WSEG_D4:WSEG_D4 + 512])
            # second half of x chunk 1 rides the ACT ring
            nc.scalar.dma_start(out=xbf[1][:, 512:N],
                                in_=x.ap()[128:256, 512:N])

            ones_bf = pp.tile([1, 512], bf16, tag="ones_bf")
            nc.vector.memset(ones_bf, 1.0)
            identsb = pp.tile([128, 128], bf16, tag="ident")
            nc.scalar.dma_start(out=identsb, in_=ident_dram.ap())
            # tiny dummy exp: hoists the exp act-table load to kernel start
            tbl = np_pool.tile([1, 8], f32, tag="tbl", name="tbl")
            nc.vector.memset(tbl, 0.0)
            nc.scalar.activation(tbl, tbl, Exp)

            for k in range(2):  # direct (d4) weights heads 2-3
                nc.scalar.dma_start(
                    out=wd4b[k],
                    in_=(wA if k == 0 else wB).ap()[:, WSEG_D4 + 512:WSEG_V])
            for k in range(2):  # compact q/k (m=1) weight segments
                nc.scalar.dma_start(
                    out=wqk[k], in_=(wA if k == 0 else wB).ap()[:, 0:WSEG_D4])

            sstsb = None
            if structured:  # four head-pair tiles on the gpsimd SWDGE ring
                sstsb = [pp.tile([128, 2 * SW], bf16, tag=f"sst{g}",
                                 name=f"sst{g}") for g in range(4)]
                nc.gpsimd.dma_start(out=sstsb[0],
                                    in_=sst.ap()[:, 0:2 * SW])
            for k in range(2):  # V weight segment (needed from ~t+20us)
                nc.gpsimd.dma_start(
                    out=wv[k], in_=(wA if k == 0 else wB).ap()[:, WSEG_V:WSEG_WO])
            if structured:
                for g in range(1, 4):
                    nc.gpsimd.dma_start(
                        out=sstsb[g],
                        in_=sst.ap()[:, 2 * g * SW:2 * (g + 1) * SW])
            for k in range(2):  # wo weight segment (needed only for the tail)
                nc.gpsimd.dma_start(
                    out=wwo[k], in_=(wA if k == 0 else wB).ap()[:, WSEG_WO:])

            # PE warmup: dummy matmuls ramp the HAM clock gate to full speed
            # and keep it warm (no >5us idle) until the input DMAs land.
            # The tile shares the "ou" tag/banks (disjoint lifetime).
            warm = ps8.tile([128, 512], f32, tag="ou", bufs=4, name="warm")
            for _ in range(10):
                nc.tensor.matmul(warm, lhsT=ones_bf[:, 0:128],
                                 rhs=ones_bf[:, 0:512], start=True, stop=True)

            # ---- compact Q/K projections + DRAM-bounce replication (h2-7),
            # direct replicated projection for heads 0-1 ----
            qsb = [pp.tile([128, N], bf16, tag=f"qsb{h}", name=f"qsb{h}")
                   for h in range(HEADS)]
            ksb = [pp.tile([128, N], bf16, tag=f"ksb{h}", name=f"ksb{h}")
                   for h in range(HEADS)]
            qcomp = pp.tile([128, N], bf16, tag="qc", name="qc")
            kcomp = pp.tile([128, N], bf16, tag="kc", name="kc")
            qdram = dp.tile([128, N], bf16, tag="qd", bufs=1, name="qd")
            kdram = dp.tile([128, N], bf16, tag="kd", bufs=1, name="kd")

            def emit_compact_proj():
                # m=1 chunk only (heads 4-7); heads 0-3 project directly
                for ten in range(2):  # 0 = Q, 1 = K
                    ps = ps8.tile([128, N], f32, tag="ps", name="ps")
                    for nt in range(NIT):
                        for k in range(2):
                            nc.tensor.matmul(
                                ps[:, nt * 512:(nt + 1) * 512],
                                lhsT=wqk[k][:, ten * 128:(ten + 1) * 128],
                                rhs=xbf[k][:, nt * 512:(nt + 1) * 512],
                                start=(k == 0), stop=(k == 1),
                            )
                    dst, dram = (qcomp, qdram) if ten == 0 else (kcomp, kdram)
                    nc.vector.tensor_scalar_add(dst, ps,
                                                offssb[:, ten:ten + 1])
                    nc.sync.dma_start(out=dram, in_=dst)

            def emit_direct_proj(h):
                """4x row-group-replicated Q/K projection for head h."""
                wd4 = wd4a if h < 2 else wd4b
                for ten in range(2):  # 0 = Q, 1 = K
                    ps = ps8.tile([128, N], f32, tag="ps", name="ps")
                    base = (2 * (h % 2) + ten) * 128
                    for nt in range(NIT):
                        for k in range(2):
                            nc.tensor.matmul(
                                ps[:, nt * 512:(nt + 1) * 512],
                                lhsT=wd4[k][:, base:base + 128],
                                rhs=xbf[k][:, nt * 512:(nt + 1) * 512],
                                start=(k == 0), stop=(k == 1),
                            )
                    col = 2 + 2 * h + ten
                    dst = qsb[h] if ten == 0 else ksb[h]
                    nc.vector.tensor_scalar_add(dst, ps,
                                                offssb[:, col:col + 1])

            def emit_replication(h, engine):
                for (dram, dst4) in ((qdram, qsb), (kdram, ksb)):
                    engine.dma_start(
                        out=dst4[h],
                        in_=bass.AP(tensor=dram.tensor,
                                    offset=dram.offset + (h - 4) * 32 * N,
                                    ap=[[0, 4], [N, 32], [1, N]]),
                    )

            # ---- attention: one-head-lag software pipeline ----
            vtsb = [pp.tile([128, VTW], bf16, tag=f"vtsb{j}", name=f"vtsb{j}")
                    for j in range(NJT)]
            gsb = [pp.tile([128, N], bf16, tag=f"gsb{t}", name=f"gsb{t}")
                   for t in range(4)]

            ets = {}          # (h, j) -> et2 tile
            ous = {}          # h -> {it: psum tile}
            state = {"last_exp": None}

            def emit_dots_pair(h, jj):
                """4 row-group-packed dots matmuls for j-tiles jj, jj+1."""
                dts = {}
                bts = {}
                for sj in range(2):
                    j = jj + sj
                    dts[j] = ps8.tile([128, N], f32, tag="ps",
                                      name=f"dots{h}_{j}")
                    if not structured:
                        bts[j] = bp.tile([128, N], bf16, tag="bt", name="bt")
                        base = (h * NJT + j) * 128
                        nc.sync.dma_start(out=bts[j],
                                          in_=sst.ap()[base:base + 128, :])
                for sj in range(2):
                    j = jj + sj
                    for it in range(NIT):
                        sl = slice(it * 512, (it + 1) * 512)
                        rg = 32 * ((2 * j + it) % 4)
                        nc.tensor.matmul(
                            dts[j][:, sl],
                            lhsT=ksb[h][rg:rg + 32, j * 128:(j + 1) * 128],
                            rhs=qsb[h][rg:rg + 32, sl],
                            start=True, stop=False,
                            tile_position=(rg, 0),
                        )
                for sj in range(2):
                    j = jj + sj
                    if structured:
                        off = (h % 2) * SW + (31 - 4 * j) * 32
                        bias_ap = sstsb[h // 2][:, off:off + N]
                    else:
                        bias_ap = bts[j]
                    for it in range(NIT):
                        sl = slice(it * 512, (it + 1) * 512)
                        nc.tensor.matmul(
                            dts[j][:, sl],
                            lhsT=identsb,
                            rhs=bias_ap[:, sl],
                            start=False, stop=True,
                        )
                for sj in range(2):
                    j = jj + sj
                    et2 = e2p.tile([128, N], bf16, tag="et2", name="et2")
                    state["last_exp"] = nc.scalar.activation(et2, dts[j], Exp)
                    ets[(h, j)] = et2

            def emit_ou_pair(h, jj):
                """OU accumulation matmuls for j-tiles jj, jj+1 of head h."""
                if jj == 0:
                    ous[h] = {it: ps8.tile([65, 512], f32, tag="ou", bufs=4,
                                           name=f"ou{h}_{it}")
                              for it in range(NIT)}
                for sj in range(2):
                    j = jj + sj
                    for it in range(NIT):
                        sl = slice(it * 512, (it + 1) * 512)
                        nc.tensor.matmul(
                            ous[h][it],
                            lhsT=vtsb[j][:, h * 65:h * 65 + 65],
                            rhs=ets[(h, j)][:, sl],
                            start=(j == 0), stop=(j == NJT - 1),
                        )

            rbs = {}

            def emit_normalize_a(h):
                """Launch the 1/S chain for head h: S -> SBUF (DVE copies),
                DRAM bounce into a [128, 8] gather so the DVE reciprocal is
                partition-wide (free-size 8, ~70ns -- NOT free-size 512),
                scatter back and broadcast-load.  The ~9us DMA latency is
                hidden: the G-multiply runs one block later (phase b)."""
                sx = np_pool.tile([1, N], f32, tag="sx", name="sx")
                for it in range(NIT):
                    nc.vector.tensor_copy(sx[:, it * 512:(it + 1) * 512],
                                          ous[h][it][64:65, :])
                rd = dp.tile([1, N], f32, tag="rd", name="rd")
                nc.sync.dma_start(out=rd, in_=sx)
                s8 = np_pool.tile([128, 8], f32, tag="s8", name="s8")
                nc.sync.dma_start(
                    out=s8,
                    in_=bass.AP(tensor=rd.tensor, offset=rd.offset,
                                ap=[[8, 128], [1, 8]]),
                )
                r8 = np_pool.tile([128, 8], bf16, tag="r8", name="r8")
                with nc.allow_low_precision(
                        reason="1/S in bf16: 0.4% on a 2e-2 gate"):
                    nc.vector.reciprocal(r8, s8)
                rd2 = dp.tile([1, N], bf16, tag="rd2", name="rd2")
                nc.sync.dma_start(
                    out=bass.AP(tensor=rd2.tensor, offset=rd2.offset,
                                ap=[[8, 128], [1, 8]]),
                    in_=r8,
                )
                rb = np_pool.tile([64, N], bf16, tag="rb", name="rb")
                nc.sync.dma_start(
                    out=rb,
                    in_=bass.AP(tensor=rd2.tensor, offset=rd2.offset,
                                ap=[[0, 64], [1, N]]),
                )
                rbs[h] = rb

            def emit_normalize_b(h):
                """G = OU * (1/S) into gsb; rb arrived a block ago."""
                rb = rbs.pop(h)
                for it in range(NIT):
                    sl = slice(it * 512, (it + 1) * 512)
                    nc.vector.tensor_mul(
                        gsb[h // 2][64 * (h % 2):64 * (h % 2) + 64, sl],
                        ous[h][it][0:64, :], rb[:, sl])
                del ous[h]

            def emit_v_pair(jj):
                """V^T projection j-tiles jj, jj+1; BN offsets and the
                softmax-denominator ones column folded in as a K=1
                ones-row matmul."""
                for j in (jj, jj + 1):
                    ps = ps8.tile([128, N], f32, tag="ps", name="vps")
                    for (lo, hi) in ((0, 512), (512, VTW)):
                        for k in range(2):
                            nc.tensor.matmul(
                                ps[:, lo:hi],
                                lhsT=xbf[k][:, j * 128:(j + 1) * 128],
                                rhs=wv[k][:, lo:hi],
                                start=(k == 0), stop=False,
                            )
                        nc.tensor.matmul(
                            ps[:, lo:hi],
                            lhsT=ones_bf[:, 0:128],
                            rhs=rowcsb[:, lo:hi],
                            start=False, stop=True,
                        )
                    nc.vector.tensor_copy(vtsb[j], ps[:, 0:VTW])

            # head 0 + startup: direct-proj head 0, first dots pair ASAP,
            # then slot the remaining projections (direct h1, compact m0/m1
            # for the h2-7 bounce, V pairs) behind dots pairs on the PE
            emit_direct_proj(0)
            emit_dots_pair(0, 0)
            emit_direct_proj(1)
            emit_dots_pair(0, 2)
            emit_direct_proj(2)
            emit_dots_pair(0, 4)
            emit_direct_proj(3)
            emit_compact_proj()
            for h in range(4, HEADS):
                emit_replication(h, nc.sync)
            emit_dots_pair(0, 6)
            emit_v_pair(0)
            emit_v_pair(2)
            emit_v_pair(4)
            emit_v_pair(6)

            # heads 1..7: dots(h) interleaved with OU(h-1); OU(7) trails.
            # normalize phases at block starts: b(h-3) multiplies with the
            # rb that arrived last block; a(h-2) launches the next chain.
            for h in range(1, HEADS):
                if h >= 3:
                    emit_normalize_b(h - 3)
                if h >= 2:
                    emit_normalize_a(h - 2)
                for jj in range(0, NJT, 2):
                    emit_dots_pair(h, jj)
                    emit_ou_pair(h - 1, jj)
            emit_normalize_a(HEADS - 2)
            emit_normalize_b(HEADS - 3)
            for jj in range(0, NJT, 2):
                emit_ou_pair(HEADS - 1, jj)
            emit_normalize_a(HEADS - 1)
            emit_normalize_b(HEADS - 2)
            emit_normalize_b(HEADS - 1)

            # ---- batched exact GELU (single act-table switch).  Gate the
            # first gelu on the last exp so the scheduler can't hoist a gelu
            # into a mid-stream ACT idle slot (act-table thrash). ----
            from concourse.tile import add_dep_helper
            for t in range(4):
                gi = nc.scalar.activation(gsb[t], gsb[t], Gelu)
                if state["last_exp"] is not None:
                    add_dep_helper(gi.ins, state["last_exp"].ins, sync=True,
                                   reason="single gelu table switch")

            # ---- output conv; conv bias+BN offset folded as ones-matmul.
            # k=0..2 partial sums for the first two groups run while
            # normalize(7)'s broadcast is still in flight; the k=3 matmuls
            # (gated on the last gelu) come last per group. ----
            groups = [(m, nt) for m in range(2) for nt in range(NIT)]
            cpss = {}
            for (m, nt) in groups[:2]:
                sl = slice(nt * 512, (nt + 1) * 512)
                cps = cpss[(m, nt)] = ps8.tile([128, N], f32, tag="ps",
                                               name="cps")
                for k in range(3):
                    nc.tensor.matmul(
                        cps[:, 0:512],
                        lhsT=wwo[k // 2][:, (k % 2) * 256 + m * 128:
                                         (k % 2) * 256 + (m + 1) * 128],
                        rhs=gsb[k][:, sl],
                        start=(k == 0), stop=False,
                    )
            for gi_, (m, nt) in enumerate(groups):
                sl = slice(nt * 512, (nt + 1) * 512)
                if (m, nt) in cpss:
                    cps = cpss[(m, nt)]
                else:
                    cps = ps8.tile([128, N], f32, tag="ps", name="cps")
                    for k in range(3):
                        nc.tensor.matmul(
                            cps[:, 0:512],
                            lhsT=wwo[k // 2][:, (k % 2) * 256 + m * 128:
                                             (k % 2) * 256 + (m + 1) * 128],
                            rhs=gsb[k][:, sl],
                            start=(k == 0), stop=False,
                        )
                nc.tensor.matmul(
                    cps[:, 0:512],
                    lhsT=wwo[1][:, 256 + m * 128:256 + (m + 1) * 128],
                    rhs=gsb[3][:, sl],
                    start=False, stop=False,
                )
                nc.tensor.matmul(
                    cps[:, 0:512],
                    lhsT=rowcsb[:, VTW + m * 128:VTW + (m + 1) * 128],
                    rhs=ones_bf[:, 0:512],
                    start=False, stop=True,
                )
                ysb = np_pool.tile([128, 512], bf16, tag="ysb", name="ysb")
                nc.vector.tensor_copy(ysb, cps[:, 0:512])
                nc.sync.dma_start(
                    out=out.ap()[m * 128:(m + 1) * 128, sl], in_=ysb)

    _split_excess_waits(nc, mybir)
    return nc


def _fold_inputs(inp):
    """Host-side BN/scale folding + compact bias table construction."""
    f32 = np.float32
    bfc = ml_dtypes.bfloat16
    scale = DK ** -0.5
    x = np.asarray(inp["x"], f32)

    def bn_fold(w, gam, bet, mu, var, s=1.0):
        inv = np.asarray(gam, f32) / np.sqrt(np.asarray(var, f32) + EPS)
        wf = (np.asarray(w, f32) * inv[:, None] * s).T.copy()        # (C, O)
        off = (np.asarray(bet, f32) - np.asarray(mu, f32) * inv) * s  # (O,)
        return wf, off

    wqt, qoff = bn_fold(inp["wq"], inp["qgam"], inp["qbet"], inp["qmu"],
                        inp["qvar"], scale)
    wkt, koff = bn_fold(inp["wk"], inp["kgam"], inp["kbet"], inp["kmu"],
                        inp["kvar"])
    wvt0, voff = bn_fold(inp["wv"], inp["vgam"], inp["vbet"], inp["vmu"],
                         inp["vvar"])

    wvt = np.zeros((C, VTW), f32)
    voffi = np.zeros(VTW, f32)
    for h in range(HEADS):
        wvt[:, 65 * h:65 * h + 64] = wvt0[:, 64 * h:64 * h + 64]
        voffi[65 * h:65 * h + 64] = voff[64 * h:64 * h + 64]
        voffi[65 * h + 64] = 1.0   # ones column -> softmax denominator

    oinv = np.asarray(inp["ogam"], f32) / np.sqrt(np.asarray(inp["ovar"], f32) + EPS)
    wot = (np.asarray(inp["wo"], f32) * oinv[:, None]).T.copy()       # (512, 256)
    ooff_v = (np.asarray(inp["bo"], f32) * oinv
              + np.asarray(inp["obet"], f32) - np.asarray(inp["omu"], f32) * oinv)

    # combined weight tensors wA (k-chunk 0) / wB (k-chunk 1):
    # [ q_m1 | k_m1 | d4 heads 0-3 (q4_h | k4_h) | wvt | wot_a | wot_b ]
    wAB = np.zeros((2, 128, WAF), f32)
    for k in range(2):
        ck = slice(k * 128, (k + 1) * 128)
        wAB[k, :, 0:128] = wqt[ck, 128:256]
        wAB[k, :, 128:256] = wkt[ck, 128:256]
        wAB[k, :, WSEG_V:WSEG_V + VTW] = wvt[ck]
        wAB[k, :, WSEG_WO:WSEG_WO + 256] = wot[(2 * k) * 128:(2 * k) * 128 + 128]
        wAB[k, :, WSEG_WO + 256:WSEG_WO + 512] = wot[(2 * k + 1) * 128:
                                                     (2 * k + 1) * 128 + 128]
        for h in range(4):
            wAB[k, :, WSEG_D4 + (2 * h) * 128:WSEG_D4 + (2 * h + 1) * 128] = \
                np.tile(wqt[ck, 32 * h:32 * h + 32], (1, 4))
            wAB[k, :, WSEG_D4 + (2 * h + 1) * 128:WSEG_D4 + (2 * h + 2) * 128] = \
                np.tile(wkt[ck, 32 * h:32 * h + 32], (1, 4))

    offs = np.zeros((128, 12), f32)
    offs[:, 0] = qoff[128:]
    offs[:, 1] = koff[128:]
    for h in range(4):
        offs[:, 2 + 2 * h] = np.tile(qoff[32 * h:32 * h + 32], 4)
        offs[:, 3 + 2 * h] = np.tile(koff[32 * h:32 * h + 32], 4)

    rowc = np.zeros((1, VTW + C), f32)
    rowc[0, :VTW] = voffi
    rowc[0, VTW:] = ooff_v

    pe = np.asarray(inp["pos_emb"], f32)             # (1024, 8)
    pidx = np.asarray(inp["pos_indices"])            # (1024, 1024) int32

    r = np.arange(F)
    pos = np.stack(np.meshgrid(r, r, indexing="ij"), axis=-1).reshape(-1, 2)
    rel = np.abs(pos[:, None, :] - pos[None, :, :])
    expected = (rel[..., 0] * F + rel[..., 1]).astype(pidx.dtype)
    structured = bool(np.array_equal(pidx, expected))

    if structured:
        dd = np.arange(63)
        xi_ = np.maximum(dd - 31, 0)
        xj_ = np.maximum(31 - dd, 0)
        yy = np.arange(F)
        I = xi_[:, None] * F + yy[None, :]           # (63, yi)
        J = xj_[:, None] * F + yy[None, :]           # (63, yj)
        idx = pidx[I[:, None, :], J[:, :, None]]     # (dd, yj, yi)
        sb = pe[idx] / scale                         # (dd, yj, yi, H)
        flat = np.ascontiguousarray(sb.transpose(3, 1, 0, 2)).reshape(HEADS, 32, 2016)
        eflat = flat   # RAW bias: added in PSUM, exp(dots + bias) fused
        sstv = np.zeros((128, HEADS * SW), f32)
        for g in range(4):
            for h in range(HEADS):
                sstv[32 * g:32 * g + 32,
                     h * SW + 32 * g:h * SW + 32 * g + 2016] = eflat[h]
    else:
        biasT = (pe[pidx] / scale).transpose(2, 1, 0)  # (H, j, i)
        sstv = biasT.reshape(HEADS * NJT * 128, N)
    sstv = sstv.astype(bfc)

    common = dict(wA=wAB[0].astype(bfc), wB=wAB[1].astype(bfc),
                  offs=offs, rowc=rowc.astype(bfc), sst=sstv)
    in_maps = []
    for b in range(B):
        m = dict(common)
        m["x"] = np.ascontiguousarray(x[b].reshape(C, N)).astype(bfc)
        in_maps.append(m)
    return in_maps, structured


def run(inputs, trace=False, trace_cores=None):
    in_maps, structured = _fold_inputs(inputs)
    key = ("nc", structured)
    if key not in _PROGRAM_CACHE:
        _PROGRAM_CACHE[key] = build_program(structured)
    nc = _PROGRAM_CACHE[key]
    from concourse.bass_utils import run_bass_kernel_spmd
    res = run_bass_kernel_spmd(
        nc, in_maps, core_ids=list(range(B)), trace=trace, trace_cores=trace_cores
    )
    out = np.stack([np.asarray(res.results[b]["out"], dtype=np.float32)
                    for b in range(B)], axis=0)
    return out.reshape(B, C, F, F), res


def kernel(**inputs):
    out, _ = run(inputs, trace=False)
    return out


# revision 49
# speedup vs baseline: 1.0317x; 1.0059x over previous
"""Trainium2 Bass kernel for nn_Attention_66907000537586 (v11).

Module: x -> 1x1conv+BN (Q,K,V) -> 8-head attention with relative position
bias -> exact GELU -> 1x1conv+bias+BN.  Shapes: B=8, C=256, F=32 (n=1024
tokens), H=8, DK=32, DV=64.

Sharding: pure data-parallel over batch (one batch element per NeuronCore).

Design (evolved from the ~185us v1 baseline through perfetto-trace
iterations):
  * ScalarE's exp stream is the hard floor: 64 x [128,1024] exp tiles at
    ~1.06us each.  Everything else is scheduled to keep that stream dense.
  * The relative-position bias is added INTO PSUM by identity-matmuls on
    the PE (which has slack), so exp computes et2 = exp(dots + bias) in a
    single ScalarE op.  The DVE is left with only softmax-normalize phases
    and projection evacuations (the earlier exp(dots)*exp(bias) form made
    the DVE a co-bottleneck; offloading to GpSimd/Pool backfires because
    Pool shares its SBUF ports with the DVE).
  * One-head-lag software pipeline: PE stream per head block is
    [dots(h) pair + bias-adds | OU(h-1) pair] x4, so dots->exp keeps ACT
    fed while OU(h-1) consumes last block's attention weights, and the PE
    never idles long enough for the HAM clock gate to re-throttle.
  * Softmax: V^T carries a ones column so OU row 64 accumulates the
    denominator S.  1/S: DVE copies S out of PSUM, a DRAM bounce gathers
    it to [128,8] (DVE reciprocal is free-size-paced: [1,512] costs 3.3us,
    [128,8] costs 70ns), reciprocal, scatter + partition-broadcast load.
    The ~9us chain latency is hidden by splitting normalize into phase a
    (launch) and phase b (G-multiply) two head-blocks later.
  * Q/K for the dots matmuls are 4x row-group-replicated so 4 K=32
    matmuls pack concurrently via tile_position.  Heads 0-1 are projected
    directly with host-replicated weight columns (early start); heads 2-7
    via a compact projection + DRAM-bounce broadcast DMAs (no engine time).
  * Input DMAs are split across the SP ring, ACT ring and gpsimd SWDGE
    with per-segment weight tiles (tile-granular DMA deps) so nothing
    waits on unrelated segments; PE warmup dummy matmuls ramp the HAM
    clock gate while DMAs are in flight; a dummy exp hoists the act-table
    load to kernel start.
  * V-BN offsets (+ ones column) and the output conv bias+BN offset are
    folded into the matmuls as K=1 ones-row matmuls.
  * Exact GELU is batched once at the end (gated on the last exp so the
    scheduler cannot hoist it into a mid-stream ACT slot, which would
    thrash the activation table); output is written bf16, upcast on host.
"""

import numpy as np
import ml_dtypes

HEADS, DK, DV, F = 8, 32, 64, 32
C = 256
N = F * F            # 1024 tokens
B = 8
EPS = 1e-5
IDK = HEADS * DK     # 256
IDV = HEADS * DV     # 512
VTW = HEADS * (DV + 1)   # 520
SW = 2112            # per-head width of the shifted compact bias table
NJT = N // 128       # 8 j-tiles
NIT = N // 512       # 2 i-tiles
WSEG_D4 = 512        # wA/wB layout: [qk m0+m1 | d4 h0-1 | v | wo]
WSEG_V = 1024
WSEG_WO = 1544
WAF = 2056

_PROGRAM_CACHE = {}


def _split_excess_waits(nc, mybir, limit=1):
    """Two post-passes over the scheduled BIR:

    1. Drop PE->PE self-semaphore waits from PE instructions (they defeat
       tile_position row-group concurrency; every PSUM-slot reuse is
       already guarded by the consumer engine's wait).
    2. Move excess semaphore sync-waits (>limit) onto carrier NoOps."""
    k = 0
    for fn in nc.m.functions:
        for bb in fn.blocks:
            out = []
            for inst in bb.instructions:
                si = inst.sync_info
                if (si is not None and si.on_wait
                        and str(inst.engine) == "EngineType.PE"
                        and type(inst).__name__ in ("InstMatmult", "InstLdweights")):
                    kept = [w for w in si.on_wait
                            if not str(w.ant_name).startswith("PE_")]
                    if len(kept) != len(si.on_wait):
                        si.on_wait = kept
                waits = list(si.on_wait) if si is not None else []
                if len(waits) > limit:
                    extra, keep = waits[:-limit], waits[-limit:]
                    for i in range(0, len(extra), limit):
                        nop = mybir.InstNoOp(name=f"waitsplit_{k}")
                        k += 1
                        nop.engine = inst.engine
                        nop.sync_info = mybir.SyncInfo(
                            on_wait=extra[i:i + limit], on_update=[])
                        out.append(nop)
                    si.on_wait = keep
                out.append(inst)
            bb.instructions = out


def build_program(structured=True):
    """Build the single-core Bass program (run SPMD on 8 cores)."""
    import concourse.bass as bass
    import concourse.mybir as mybir
    import concourse.tile as tile
    from concourse.tile import add_dep_helper

    dt = mybir.dt
    nc = bass.Bass("TRN2", target_bir_lowering=False, debug=False, num_devices=B)

    f32, bf16 = dt.float32, dt.bfloat16
    Exp = mybir.ActivationFunctionType.Exp
    Gelu = mybir.ActivationFunctionType.Gelu

    x = nc.dram_tensor("x", [C, N], bf16, kind="ExternalInput")
    wA = nc.dram_tensor("wA", [128, WAF], bf16, kind="ExternalInput")
    wB = nc.dram_tensor("wB", [128, WAF], bf16, kind="ExternalInput")
    offs = nc.dram_tensor("offs", [128, 8], f32, kind="ExternalInput")
    rowc = nc.dram_tensor("rowc", [1, VTW + C], bf16, kind="ExternalInput")
    if structured:
        sst = nc.dram_tensor("sst", [128, HEADS * SW], bf16, kind="ExternalInput")
    else:
        sst = nc.dram_tensor("sst", [HEADS * NJT * 128, N], bf16, kind="ExternalInput")
    out = nc.dram_tensor("out", [C, N], bf16, kind="ExternalOutput")
    ident_dram = nc.inline_tensor(np.eye(128, dtype=ml_dtypes.bfloat16),
                                  name="ident128")

    with tile.TileContext(nc) as tc:
        with (
            tc.tile_pool(name="persist", bufs=1) as pp,
            tc.tile_pool(name="exps2", bufs=12) as e2p,
            tc.tile_pool(name="norm", bufs=2) as np_pool,
            tc.tile_pool(name="bias_stream", bufs=4) as bp,
            tc.tile_pool(name="dramscratch", bufs=2, space="DRAM") as dp,
            tc.tile_pool(name="ps8", bufs=2, space="PSUM") as ps8,
        ):
            # ---- input DMAs.  SP ring: x chunk 0 + x1 first half, then
            # bounce/replication/normalize traffic.  ACT ring: offsets,
            # d4 weights, x1 second half, compact qk weights.  gpsimd
            # SWDGE: bias table, V and wo weights, h4-7 replication. ----
            xbf = [pp.tile([128, N], bf16, tag=f"xbf{k}", name=f"xbf{k}")
                   for k in range(2)]
            nc.sync.dma_start(out=xbf[0], in_=x.ap()[0:128, :])
            nc.sync.dma_start(out=xbf[1][:, 0:512], in_=x.ap()[128:256, 0:512])
            rowcsb = pp.tile([1, VTW + C], bf16, tag="rowc")
            nc.sync.dma_start(out=rowcsb, in_=rowc.ap())

            wqk = [pp.tile([128, 512], bf16, tag=f"wqk{k}", name=f"wqk{k}")
                   for k in range(2)]
            wd4 = [pp.tile([128, 512], bf16, tag=f"wd4{k}", name=f"wd4{k}")
                   for k in range(2)]
            wv = [pp.tile([128, VTW], bf16, tag=f"wv{k}", name=f"wv{k}")
                  for k in range(2)]
            wwo = [pp.tile([128, 512], bf16, tag=f"wwo{k}", name=f"wwo{k}")
                   for k in range(2)]
            offssb = pp.tile([128, 8], f32, tag="offs")
            nc.scalar.dma_start(out=offssb, in_=offs.ap())
            identsb = pp.tile([128, 128], bf16, tag="ident")
            nc.scalar.dma_start(out=identsb, in_=ident_dram.ap())
            for k in range(2):  # direct (d4) weights
                nc.scalar.dma_start(
                    out=wd4[k],
                    in_=(wA if k == 0 else wB).ap()[:, WSEG_D4:WSEG_V])
            # second half of x chunk 1 rides the ACT ring
            nc.scalar.dma_start(out=xbf[1][:, 512:N],
                                in_=x.ap()[128:256, 512:N])

            ones_bf = pp.tile([1, 512], bf16, tag="ones_bf")
            nc.vector.memset(ones_bf, 1.0)
            # tiny dummy exp: hoists the exp act-table load to kernel start
            tbl = np_pool.tile([1, 8], f32, tag="tbl", name="tbl")
            nc.vector.memset(tbl, 0.0)
            nc.scalar.activation(tbl, tbl, Exp)

            for k in range(2):  # compact q/k weight segments
                nc.scalar.dma_start(
                    out=wqk[k], in_=(wA if k == 0 else wB).ap()[:, 0:WSEG_D4])

            sstsb = None
            if structured:  # four head-pair tiles on the gpsimd SWDGE ring
                sstsb = [pp.tile([128, 2 * SW], bf16, tag=f"sst{g}",
                                 name=f"sst{g}") for g in range(4)]
                nc.gpsimd.dma_start(out=sstsb[0], in_=sst.ap()[:, 0:2 * SW])
            for k in range(2):  # V weight segment (needed from ~t+20us)
                nc.gpsimd.dma_start(
                    out=wv[k], in_=(wA if k == 0 else wB).ap()[:, WSEG_V:WSEG_WO])
            if structured:
                for g in range(1, 4):
                    nc.gpsimd.dma_start(
                        out=sstsb[g],
                        in_=sst.ap()[:, 2 * g * SW:2 * (g + 1) * SW])
            for k in range(2):  # wo weight segment (needed only for the tail)
                nc.gpsimd.dma_start(
                    out=wwo[k], in_=(wA if k == 0 else wB).ap()[:, WSEG_WO:])

            # PE warmup: dummy matmuls ramp the HAM clock gate to full speed
            # while the input DMAs are in flight (shares the "ou" banks)
            warm = ps8.tile([128, 512], f32, tag="ou", bufs=4, name="warm")
            for _ in range(10):
                nc.tensor.matmul(warm, lhsT=ones_bf[:, 0:128],
                                 rhs=ones_bf[:, 0:512], start=True, stop=True)

            # ---- Q/K projections ----
            qsb = [pp.tile([128, N], bf16, tag=f"qsb{h}", name=f"qsb{h}")
                   for h in range(HEADS)]
            ksb = [pp.tile([128, N], bf16, tag=f"ksb{h}", name=f"ksb{h}")
                   for h in range(HEADS)]
            qcomp = [pp.tile([128, N], bf16, tag=f"qc{m}", name=f"qc{m}")
                     for m in range(2)]
            kcomp = [pp.tile([128, N], bf16, tag=f"kc{m}", name=f"kc{m}")
                     for m in range(2)]
            qdram = dp.tile([C, N], bf16, tag="qd", bufs=1, name="qd")
            kdram = dp.tile([C, N], bf16, tag="kd", bufs=1, name="kd")

            def emit_compact_proj(m):
                for ten in range(2):  # 0 = Q, 1 = K
                    ps = ps8.tile([128, N], f32, tag="ps", name="ps")
                    for nt in range(NIT):
                        for k in range(2):
                            nc.tensor.matmul(
                                ps[:, nt * 512:(nt + 1) * 512],
                                lhsT=wqk[k][:, ten * 256 + m * 128:
                                            ten * 256 + (m + 1) * 128],
                                rhs=xbf[k][:, nt * 512:(nt + 1) * 512],
                                start=(k == 0), stop=(k == 1),
                            )
                    dst = (qcomp if ten == 0 else kcomp)[m]
                    dram = qdram if ten == 0 else kdram
                    nc.vector.tensor_scalar_add(dst, ps,
                                                offssb[:, 2 * ten + m:
                                                       2 * ten + m + 1])
                    nc.sync.dma_start(out=dram[m * 128:(m + 1) * 128, :],
                                      in_=dst)

            def emit_direct_proj(h):
                """4x row-group-replicated Q/K projection for head h (0-1)."""
                for ten in range(2):  # 0 = Q, 1 = K
                    ps = ps8.tile([128, N], f32, tag="ps", name="ps")
                    base = (2 * h + ten) * 128
                    for nt in range(NIT):
                        for k in range(2):
                            nc.tensor.matmul(
                                ps[:, nt * 512:(nt + 1) * 512],
                                lhsT=wd4[k][:, base:base + 128],
                                rhs=xbf[k][:, nt * 512:(nt + 1) * 512],
                                start=(k == 0), stop=(k == 1),
                            )
                    col = 4 + 2 * h + ten
                    dst = (qsb if ten == 0 else ksb)[h]
                    nc.vector.tensor_scalar_add(dst, ps,
                                                offssb[:, col:col + 1])

            def emit_replication(h, engine):
                for (dram, dst4) in ((qdram, qsb), (kdram, ksb)):
                    engine.dma_start(
                        out=dst4[h],
                        in_=bass.AP(tensor=dram.tensor,
                                    offset=dram.offset + h * 32 * N,
                                    ap=[[0, 4], [N, 32], [1, N]]),
                    )

            # ---- attention: one-head-lag software pipeline ----
            vtsb = [pp.tile([128, VTW], bf16, tag=f"vtsb{j}", name=f"vtsb{j}")
                    for j in range(NJT)]
            gsb = [pp.tile([128, N], bf16, tag=f"gsb{t}", name=f"gsb{t}")
                   for t in range(4)]

            ets = {}          # (h, j) -> et2 tile
            ous = {}          # h -> {it: psum tile}
            rbs = {}          # h -> broadcast 1/S tile
            state = {"last_exp": None}

            def emit_dots_pair(h, jj):
                """Row-group-packed dots matmuls + identity bias-adds for
                j-tiles jj, jj+1, then fused et2 = exp(dots + bias)."""
                dts = {}
                bts = {}
                for sj in range(2):
                    j = jj + sj
                    dts[j] = ps8.tile([128, N], f32, tag="ps",
                                      name=f"dots{h}_{j}")
                    if not structured:
                        bts[j] = bp.tile([128, N], bf16, tag="bt", name="bt")
                        base = (h * NJT + j) * 128
                        nc.sync.dma_start(out=bts[j],
                                          in_=sst.ap()[base:base + 128, :])
                for sj in range(2):
                    j = jj + sj
                    for it in range(NIT):
                        sl = slice(it * 512, (it + 1) * 512)
                        rg = 32 * ((2 * j + it) % 4)
                        nc.tensor.matmul(
                            dts[j][:, sl],
                            lhsT=ksb[h][rg:rg + 32, j * 128:(j + 1) * 128],
                            rhs=qsb[h][rg:rg + 32, sl],
                            start=True, stop=False,
                            tile_position=(rg, 0),
                        )
                for sj in range(2):
                    j = jj + sj
                    if structured:
                        off = (h % 2) * SW + (31 - 4 * j) * 32
                        bias_ap = sstsb[h // 2][:, off:off + N]
                    else:
                        bias_ap = bts[j]
                    for it in range(NIT):
                        sl = slice(it * 512, (it + 1) * 512)
                        nc.tensor.matmul(
                            dts[j][:, sl],
                            lhsT=identsb,
                            rhs=bias_ap[:, sl],
                            start=False, stop=True,
                        )
                for sj in range(2):
                    j = jj + sj
                    et2 = e2p.tile([128, N], bf16, tag="et2", name="et2")
                    state["last_exp"] = nc.scalar.activation(et2, dts[j], Exp)
                    ets[(h, j)] = et2

            def emit_ou_pair(h, jj):
                """OU accumulation matmuls for j-tiles jj, jj+1 of head h."""
                if jj == 0:
                    ous[h] = {it: ps8.tile([65, 512], f32, tag="ou", bufs=4,
                                           name=f"ou{h}_{it}")
                              for it in range(NIT)}
                for sj in range(2):
                    j = jj + sj
                    for it in range(NIT):
                        sl = slice(it * 512, (it + 1) * 512)
                        nc.tensor.matmul(
                            ous[h][it],
                            lhsT=vtsb[j][:, h * 65:h * 65 + 65],
                            rhs=ets[(h, j)][:, sl],
                            start=(j == 0), stop=(j == NJT - 1),
                        )

            def emit_normalize_a(h):
                """Launch the 1/S chain for head h (latency hidden: the
                G-multiply runs one block later, phase b)."""
                sx = np_pool.tile([1, N], f32, tag="sx", name="sx")
                for it in range(NIT):
                    nc.vector.tensor_copy(sx[:, it * 512:(it + 1) * 512],
                                          ous[h][it][64:65, :])
                rd = dp.tile([1, N], f32, tag="rd", name="rd")
                nc.sync.dma_start(out=rd, in_=sx)
                s8 = np_pool.tile([128, 8], f32, tag="s8", name="s8")
                nc.sync.dma_start(
                    out=s8,
                    in_=bass.AP(tensor=rd.tensor, offset=rd.offset,
                                ap=[[8, 128], [1, 8]]),
                )
                r8 = np_pool.tile([128, 8], bf16, tag="r8", name="r8")
                with nc.allow_low_precision(
                        reason="1/S in bf16: 0.4% on a 2e-2 gate"):
                    nc.vector.reciprocal(r8, s8)
                rd2 = dp.tile([1, N], bf16, tag="rd2", name="rd2")
                nc.sync.dma_start(
                    out=bass.AP(tensor=rd2.tensor, offset=rd2.offset,
                                ap=[[8, 128], [1, 8]]),
                    in_=r8,
                )
                rb = np_pool.tile([64, N], bf16, tag="rb", name="rb")
                nc.sync.dma_start(
                    out=rb,
                    in_=bass.AP(tensor=rd2.tensor, offset=rd2.offset,
                                ap=[[0, 64], [1, N]]),
                )
                rbs[h] = rb

            def emit_normalize_b(h):
                """G = OU * (1/S) into gsb; rb arrived a block ago."""
                rb = rbs.pop(h)
                for it in range(NIT):
                    sl = slice(it * 512, (it + 1) * 512)
                    nc.vector.tensor_mul(
                        gsb[h // 2][64 * (h % 2):64 * (h % 2) + 64, sl],
                        ous[h][it][0:64, :], rb[:, sl])
                del ous[h]

            def emit_v_pair(jj):
                """V^T projection j-tiles jj, jj+1; BN offsets + the ones
                column folded in as a K=1 ones-row matmul."""
                for j in (jj, jj + 1):
                    ps = ps8.tile([128, N], f32, tag="ps", name="vps")
                    for (lo, hi) in ((0, 512), (512, VTW)):
                        for k in range(2):
                            nc.tensor.matmul(
                                ps[:, lo:hi],
                                lhsT=xbf[k][:, j * 128:(j + 1) * 128],
                                rhs=wv[k][:, lo:hi],
                                start=(k == 0), stop=False,
                            )
                        nc.tensor.matmul(
                            ps[:, lo:hi],
                            lhsT=ones_bf[:, 0:128],
                            rhs=rowcsb[:, lo:hi],
                            start=False, stop=True,
                        )
                    nc.vector.tensor_copy(vtsb[j], ps[:, 0:VTW])

            # ---- startup: head-0 attention ASAP, projections threaded in
            emit_direct_proj(0)
            emit_dots_pair(0, 0)
            emit_direct_proj(1)
            emit_dots_pair(0, 2)
            emit_compact_proj(0)
            emit_dots_pair(0, 4)
            emit_compact_proj(1)
            emit_replication(2, nc.sync)
            emit_replication(3, nc.sync)
            emit_dots_pair(0, 6)
            emit_v_pair(0)
            emit_v_pair(2)
            emit_v_pair(4)
            emit_v_pair(6)
            for h in range(4, HEADS):
                emit_replication(h, nc.gpsimd)

            # heads 1..7: dots(h) interleaved with OU(h-1); OU(7) trails.
            # normalize phases at block starts: b(h-3) multiplies with the
            # rb that arrived last block; a(h-2) launches the next chain.
            for h in range(1, HEADS):
                if h >= 3:
                    emit_normalize_b(h - 3)
                if h >= 2:
                    emit_normalize_a(h - 2)
                for jj in range(0, NJT, 2):
                    emit_dots_pair(h, jj)
                    emit_ou_pair(h - 1, jj)
            emit_normalize_a(HEADS - 2)
            emit_normalize_b(HEADS - 3)
            for jj in range(0, NJT, 2):
                emit_ou_pair(HEADS - 1, jj)
            emit_normalize_a(HEADS - 1)
            emit_normalize_b(HEADS - 2)
            emit_normalize_b(HEADS - 1)

            # ---- batched exact GELU (single act-table switch, gated so the
            # scheduler cannot hoist it mid-stream) ----
            for t in range(4):
                gi = nc.scalar.activation(gsb[t], gsb[t], Gelu)
                if state["last_exp"] is not None:
                    add_dep_helper(gi.ins, state["last_exp"].ins, sync=True,
                                   reason="single gelu table switch")

            # ---- output conv; bias+BN offset folded as a ones-matmul.
            # k=0..2 partials for the first groups overlap normalize(7). ----
            groups = [(m, nt) for m in range(2) for nt in range(NIT)]
            cpss = {}

            def conv_partial(m, nt):
                sl = slice(nt * 512, (nt + 1) * 512)
                cps = ps8.tile([128, N], f32, tag="ps", name="cps")
                for k in range(3):
                    nc.tensor.matmul(
                        cps[:, 0:512],
                        lhsT=wwo[k // 2][:, (k % 2) * 256 + m * 128:
                                         (k % 2) * 256 + (m + 1) * 128],
                        rhs=gsb[k][:, sl],
                        start=(k == 0), stop=False,
                    )
                return cps

            for (m, nt) in groups[:2]:
                cpss[(m, nt)] = conv_partial(m, nt)
            for (m, nt) in groups:
                sl = slice(nt * 512, (nt + 1) * 512)
                cps = cpss.get((m, nt)) or conv_partial(m, nt)
                nc.tensor.matmul(
                    cps[:, 0:512],
                    lhsT=wwo[1][:, 256 + m * 128:256 + (m + 1) * 128],
                    rhs=gsb[3][:, sl],
                    start=False, stop=False,
                )
                nc.tensor.matmul(
                    cps[:, 0:512],
                    lhsT=rowcsb[:, VTW + m * 128:VTW + (m + 1) * 128],
                    rhs=ones_bf[:, 0:512],
                    start=False, stop=True,
                )
                ysb = np_pool.tile([128, 512], bf16, tag="ysb", name="ysb")
                nc.vector.tensor_copy(ysb, cps[:, 0:512])
                nc.sync.dma_start(
                    out=out.ap()[m * 128:(m + 1) * 128, sl], in_=ysb)

    _split_excess_waits(nc, mybir)
    return nc


def _fold_inputs(inp):
    """Host-side BN/scale folding + compact bias table construction."""
    f32 = np.float32
    bfc = ml_dtypes.bfloat16
    scale = DK ** -0.5
    x = np.asarray(inp["x"], f32)

    def bn_fold(w, gam, bet, mu, var, s=1.0):
        inv = np.asarray(gam, f32) / np.sqrt(np.asarray(var, f32) + EPS)
        wf = (np.asarray(w, f32) * inv[:, None] * s).T.copy()        # (C, O)
        off = (np.asarray(bet, f32) - np.asarray(mu, f32) * inv) * s  # (O,)
        return wf, off

    wqt, qoff = bn_fold(inp["wq"], inp["qgam"], inp["qbet"], inp["qmu"],
                        inp["qvar"], scale)
    wkt, koff = bn_fold(inp["wk"], inp["kgam"], inp["kbet"], inp["kmu"],
                        inp["kvar"])
    wvt0, voff = bn_fold(inp["wv"], inp["vgam"], inp["vbet"], inp["vmu"],
                         inp["vvar"])

    wvt = np.zeros((C, VTW), f32)
    voffi = np.zeros(VTW, f32)
    for h in range(HEADS):
        wvt[:, 65 * h:65 * h + 64] = wvt0[:, 64 * h:64 * h + 64]
        voffi[65 * h:65 * h + 64] = voff[64 * h:64 * h + 64]
        voffi[65 * h + 64] = 1.0   # ones column -> softmax denominator

    oinv = np.asarray(inp["ogam"], f32) / np.sqrt(np.asarray(inp["ovar"], f32) + EPS)
    wot = (np.asarray(inp["wo"], f32) * oinv[:, None]).T.copy()       # (512, 256)
    ooff_v = (np.asarray(inp["bo"], f32) * oinv
              + np.asarray(inp["obet"], f32) - np.asarray(inp["omu"], f32) * oinv)

    # combined weight tensors wA (k-chunk 0) / wB (k-chunk 1):
    # [ wqt | wkt | q4_0 | k4_0 | q4_1 | k4_1 | wvt | wot_a | wot_b ]
    wAB = np.zeros((2, 128, WAF), f32)
    for k in range(2):
        ck = slice(k * 128, (k + 1) * 128)
        wAB[k, :, 0:256] = wqt[ck]
        wAB[k, :, 256:512] = wkt[ck]
        for h in range(2):
            wAB[k, :, WSEG_D4 + (2 * h) * 128:WSEG_D4 + (2 * h + 1) * 128] = \
                np.tile(wqt[ck, 32 * h:32 * h + 32], (1, 4))
            wAB[k, :, WSEG_D4 + (2 * h + 1) * 128:WSEG_D4 + (2 * h + 2) * 128] = \
                np.tile(wkt[ck, 32 * h:32 * h + 32], (1, 4))
        wAB[k, :, WSEG_V:WSEG_V + VTW] = wvt[ck]
        wAB[k, :, WSEG_WO:WSEG_WO + 256] = wot[(2 * k) * 128:(2 * k) * 128 + 128]
        wAB[k, :, WSEG_WO + 256:WSEG_WO + 512] = wot[(2 * k + 1) * 128:
                                                     (2 * k + 1) * 128 + 128]

    offs = np.zeros((128, 8), f32)
    offs[:, 0] = qoff[:128]
    offs[:, 1] = qoff[128:]
    offs[:, 2] = koff[:128]
    offs[:, 3] = koff[128:]
    for h in range(2):
        offs[:, 4 + 2 * h] = np.tile(qoff[32 * h:32 * h + 32], 4)
        offs[:, 5 + 2 * h] = np.tile(koff[32 * h:32 * h + 32], 4)

    rowcv = np.zeros((1, VTW + C), f32)
    rowcv[0, :VTW] = voffi
    rowcv[0, VTW:] = ooff_v

    pe = np.asarray(inp["pos_emb"], f32)             # (1024, 8)
    pidx = np.asarray(inp["pos_indices"])            # (1024, 1024) int32

    r = np.arange(F)
    pos = np.stack(np.meshgrid(r, r, indexing="ij"), axis=-1).reshape(-1, 2)
    rel = np.abs(pos[:, None, :] - pos[None, :, :])
    expected = (rel[..., 0] * F + rel[..., 1]).astype(pidx.dtype)
    structured = bool(np.array_equal(pidx, expected))

    if structured:
        dd = np.arange(63)
        xi_ = np.maximum(dd - 31, 0)
        xj_ = np.maximum(31 - dd, 0)
        yy = np.arange(F)
        I = xi_[:, None] * F + yy[None, :]           # (63, yi)
        J = xj_[:, None] * F + yy[None, :]           # (63, yj)
        idx = pidx[I[:, None, :], J[:, :, None]]     # (dd, yj, yi)
        sb = pe[idx] / scale                         # (dd, yj, yi, H)
        flat = np.ascontiguousarray(sb.transpose(3, 1, 0, 2)).reshape(HEADS, 32, 2016)
        # RAW bias values: added in PSUM, exp(dots + bias) fused on ScalarE
        sstv = np.zeros((128, HEADS * SW), f32)
        for g in range(4):
            for h in range(HEADS):
                sstv[32 * g:32 * g + 32,
                     h * SW + 32 * g:h * SW + 32 * g + 2016] = flat[h]
    else:
        biasT = (pe[pidx] / scale).transpose(2, 1, 0)  # (H, j, i)
        sstv = biasT.reshape(HEADS * NJT * 128, N)
    sstv = sstv.astype(bfc)

    common = dict(wA=wAB[0].astype(bfc), wB=wAB[1].astype(bfc),
                  offs=offs, rowc=rowcv.astype(bfc), sst=sstv)
    in_maps = []
    for b in range(B):
        m = dict(common)
        m["x"] = np.ascontiguousarray(x[b].reshape(C, N)).astype(bfc)
        in_maps.append(m)
    return in_maps, structured


def run(inputs, trace=False, trace_cores=None):
    in_maps, structured = _fold_inputs(inputs)
    key = ("nc", structured)
    if key not in _PROGRAM_CACHE:
        _PROGRAM_CACHE[key] = build_program(structured)
    nc = _PROGRAM_CACHE[key]
    from concourse.bass_utils import run_bass_kernel_spmd
    res = run_bass_kernel_spmd(
        nc, in_maps, core_ids=list(range(B)), trace=trace, trace_cores=trace_cores
    )
    out = np.stack([np.asarray(res.results[b]["out"], dtype=np.float32)
                    for b in range(B)], axis=0)
    return out.reshape(B, C, F, F), res


def kernel(**inputs):
    out, _ = run(inputs, trace=False)
    return out
